# revision 36
# baseline (speedup 1.0000x reference)
"""Trainium2 Bass kernel for nn_ExemplarSoftmaxLoss (data-parallel over 8 cores).

Strategy:
  - Shard batch dim B (and the 3 B-row blocks of `outputs`) across 8 cores;
    host reduces the 8x[128,4] partial sums in float64.
  - Softmax: per-row sum(exp(x)) via ScalarE Exp with row-accumulate (out is
    a throwaway fp16 tile); the label-logit SUM over all 3 thirds of a
    row-block comes from ONE custom DVE pass over the [128,3,1000] x-tile:
    accum += x * (1[Idx==la] + 1[Idx==la+1000] + 1[Idx==ln+2000]).
  - Distances: the exemplar table is staged to DRAM as fp16 once (SWDGE
    cast-loads + stores through a tracked DRAM tile), then gathered per
    128-row block with [128,1]-offset indirect DMAs at half the bytes; a
    custom DVE op computes a running prefix of (a-b+eps)^2 over [128,8,512]
    supergroups in one VectorE pass; ScalarE extracts the prefix at the
    eight 512-element boundaries and the tail takes adjacent differences
    against a leading zero column.
  - Streaming: host packs `outputs` as [16,128,3,1000] and a/p/n as
    [2,128,3,8,512]; the Sync queue carries the x-tile stream with the six
    2MB apn pieces interleaved; sg1 gathers are gated on a mid-kernel label
    reload so the SWDGE descriptor ring drains instead of stalling Q7.
"""

import os
import sys

import numpy as np

for _p in ("/opt/trn_rl_repo",):
    if _p not in sys.path and os.path.isdir(_p):
        sys.path.insert(0, _p)

import concourse.bass as bass
import concourse.tile as tile
from concourse import bacc, mybir
from concourse._compat import with_exitstack
from concourse.bass_utils import run_bass_kernel_spmd

# If BASS_TRACE is set in the environment, run_bass_kernel_spmd imports
# antenv.axon_hooks, which this image lacks -- stub it so we degrade to
# an untraced run instead of crashing.
try:
    import antenv.axon_hooks  # noqa: F401
except ImportError:
    import types as _types

    _m = _types.ModuleType("antenv.axon_hooks")
    _m.get_axon_ntff_profile_hook = lambda: None
    _m.set_axon_ntff_profile_hook = lambda h: None
    sys.modules["antenv.axon_hooks"] = _m

# Problem constants (hardcoded per the harness contract).
B, D, C = 16384, 512, 1000
NCORES = 8
BS = B // NCORES  # 2048 batch rows per core
RS = 3 * BS  # 6144 softmax rows per core
P = 128
NB = BS // P  # 16 row-blocks
NR = 3 * NB  # 48 (third, row-block) pairs
NSG = 2  # supergroups of 8 row-blocks in the distance phase
SGB = NB // NSG  # 8 blocks per supergroup
EPS = 1e-6
MARGIN2 = 0.2
LAMBDA = 1.0

f32 = mybir.dt.float32
f16 = mybir.dt.float16
i32 = mybir.dt.int32
Alu = mybir.AluOpType
Act = mybir.ActivationFunctionType
AX = mybir.AxisListType

LAST_RESULTS = None  # BassKernelResults of the most recent run (for test.py)


# ---- custom DVE ops --------------------------------------------------------
def _register_op(name, spec_fn):
    from concourse import dve_ops as dvo
    from concourse.dve_spec import lower, _has_src1
    from concourse.dve_uop import DveOpSpec

    for op in dvo.OPS:
        if op.name == name:
            return op
    spec = spec_fn()
    row = max(dvo._SUB_OPCODE_FOR_NAME.values()) + 1
    assert row < 0x20
    dvo._SUB_OPCODE_FOR_NAME[name] = row
    uops = lower(spec, ver="v3")
    sha = DveOpSpec(
        name=name, opcode=row, uops=uops, rd1_en=_has_src1(spec)
    ).sha("v3")
    op = dvo.DveOp(name, spec, subdim=False, uops_sha={"v3": sha})
    dvo.OPS.append(op)
    dvo.CUSTOM_DVE_SPECS[name] = spec
    return op


def _sqdiff_spec():
    from concourse.dve_spec import AluOp, C0, Spec, Src0, Src1, scan, sq

    def _ref(in0, in1, c0, c1, c2):
        a = np.asarray(in0, np.float32).reshape(in0.shape[0], -1)
        b = np.asarray(in1, np.float32).reshape(in0.shape[0], -1)
        c0v = c0 if isinstance(c0, float) else np.asarray(c0, np.float32)
        d = a - b + c0v
        return np.cumsum(d * d, axis=1)

    return Spec(body=scan(AluOp.ADD, sq(Src0 - Src1 + C0)), reference=_ref)


def _label3_spec():
    from operator import add

    from concourse.dve_spec import C0, C1, C2, Idx, Spec, Src0, Zero, eq

    # accum += x * (1[k==la] + 1[k==la+1000] + 1[k==ln+2000])
    # s0 = la, s1 = ln + 2000 (host-computed), imm2 = 1000.0
    body = Src0 * (eq(Idx, C0) + eq(Idx, C0 + C2) + eq(Idx, C1))

    def _ref(in0, in1, c0, c1, c2):
        x = np.asarray(in0, np.float32).reshape(in0.shape[0], -1)
        k = np.arange(x.shape[1], dtype=np.float32)[None, :]
        m = (k == c0) + (k == c0 + c2) + (k == c1)
        out = x * m
        return out, out.sum(axis=1, keepdims=True)

    return Spec(body=body, accum=add, accum_init=Zero, reference=_ref)


SQDIFF_OP = _register_op("SQDIFF_PREFIX_ANT", _sqdiff_spec)
LABEL3_OP = _register_op("LABEL3_EXTRACT_ANT", _label3_spec)


@with_exitstack
def _emit(ctx, tc, outs, ins):
    nc = tc.nc
    xo = ins["xout"]  # [NB, P, 3, C] f32  outputs, host-packed tiles
    apn = ins["apn"]  # [NSG, P, 3, SGB, D] f32  anchor/positive/negative
    ex = ins["exem"]  # [C, D] f32  exemplar table
    la = ins["lab_a"]  # [P, NB] i32  labels_anchor, row blk*128+p at [p, blk]
    ln_ = ins["lab_n"]  # [P, NB] i32  labels_neg
    laf = ins["laf"]  # [P, NB] f32  labels_anchor as f32
    lnf = ins["lnf"]  # [P, NB] f32  labels_neg + 2000 as f32
    pd = outs["partials"]  # [P, 4] f32

    sing = ctx.enter_context(tc.tile_pool(name="sing", bufs=1))
    xpool = ctx.enter_context(tc.tile_pool(name="xp", bufs=4))
    ejp = ctx.enter_context(tc.tile_pool(name="ejp", bufs=1))
    mxp = ctx.enter_context(tc.tile_pool(name="mxp", bufs=1))
    apnp = ctx.enter_context(tc.tile_pool(name="apnp", bufs=2))
    prefp = ctx.enter_context(tc.tile_pool(name="prefp", bufs=1, space="PSUM"))
    exdp = ctx.enter_context(tc.tile_pool(name="exdp", bufs=1, space="DRAM"))
    exsp = ctx.enter_context(tc.tile_pool(name="exsp", bufs=1))

    sums = sing.tile([P, NR], f32)  # per-(third,block) sum(exp(x))
    lblX = sing.tile([P, NB], f32)  # per-block sum of 3 label logits
    # prefix extracts, pair-major; col 0 of each supergroup is a zero so the
    # tail can take adjacent differences with one subtract
    d2p = sing.tile([P, 6, NSG, SGB + 1], f32)
    la_t = sing.tile([P, NB], i32)
    ln_t = sing.tile([P, NB], i32)
    laf_t = sing.tile([P, NB], f32)
    lnf_t = sing.tile([P, NB], f32)
    exa = sing.tile([P, NB, D], f16)  # gathered exemplars[labels_anchor]
    exn = sing.tile([P, NB, D], f16)  # gathered exemplars[labels_neg]
    la2_t = sing.tile([P, SGB], i32)  # sg1 gather gate: reloaded mid-kernel
    ln2_t = sing.tile([P, SGB], i32)
    nc.gpsimd.memset(d2p[:], 0.0)

    # small loads via SWDGE so the Sync HWDGE queue leads with the x-tile stream
    nc.gpsimd.dma_start(out=la_t[:], in_=la[:])
    nc.gpsimd.dma_start(out=ln_t[:], in_=ln_[:])
    nc.gpsimd.dma_start(out=laf_t[:], in_=laf[:])
    nc.gpsimd.dma_start(out=lnf_t[:], in_=lnf[:])

    exf = exdp.tile([C, D], f16, name="exf16")
    exs_a = exsp.tile([P, 7, D], f16, tag="exs_a", name="exs_a")
    exs_b = exsp.tile([P, D], f16, tag="exs_b", name="exs_b")
    nc.gpsimd.dma_start(
        out=exs_a[:], in_=ex[0 : 7 * P, :].rearrange("(a p) d -> p a d", p=P)
    )
    nc.gpsimd.dma_start(out=exs_b[0 : C - 7 * P, :], in_=ex[7 * P :, :])
    def emit_exf_stores():
        nc.sync.dma_start(
            out=exf[0 : 7 * P, :].rearrange("(a p) d -> p a d", p=P), in_=exs_a[:]
        )
        nc.sync.dma_start(out=exf[7 * P :, :], in_=exs_b[0 : C - 7 * P, :])

    def emit_gathers(dst, lab_t, blks, col0=0):
        for blk in blks:
            nc.gpsimd.indirect_dma_start(
                out=dst[:, blk, :],
                out_offset=None,
                in_=exf[:],
                in_offset=bass.IndirectOffsetOnAxis(
                    ap=lab_t[:, blk - col0 : blk - col0 + 1], axis=0
                ),
            )

    apn_tiles = {}

    def emit_apn_load(s, part):
        # one 2MB piece per (supergroup, tensor) on the Sync queue (a SWDGE
        # cast-load would flood the Q7 descriptor ring and stall the gathers)
        if s not in apn_tiles:
            apn_tiles[s] = apnp.tile(
                [P, 3, SGB, D], f32, tag="apn", name=f"apn{s}"
            )
        t = apn_tiles[s]
        nc.sync.dma_start(out=t[:, part : part + 1], in_=apn[s, :, part : part + 1])

    def emit_xtile(i):
        xt = xpool.tile([P, 3, C], f32, tag="xt", name=f"xt{i}")
        nc.sync.dma_start(out=xt[:], in_=xo[i])
        for t in range(3):
            col = t * NB + i
            ej = ejp.tile([P, C], f16, tag="ej")
            nc.scalar.activation(
                out=ej[:],
                in_=xt[:, t, :],
                func=Act.Exp,
                accum_out=sums[:, col : col + 1],
            )
        mx = mxp.tile([P, 3, C], f16, tag="mx", name=f"mx{i}")
        nc.vector._custom_dve(
            LABEL3_OP,
            out=mx[:],
            in0=xt[:],
            s0=laf_t[:, i : i + 1],
            s1=lnf_t[:, i : i + 1],
            imm2=1000.0,
            accum_out=lblX[:, i : i + 1],
        )

    def emit_pair(s, ci):
        t = apn_tiles[s]
        a = t[:, 0]
        sl_ = slice(s * SGB, (s + 1) * SGB)
        pairs = (
            (a, exa[:, sl_, :]),  # d_ref1
            (t[:, 2], exa[:, sl_, :]),  # d_neg1
            (a, exn[:, sl_, :]),  # d_ref2
            (t[:, 2], exn[:, sl_, :]),  # d_neg2
            (a, t[:, 1]),  # tp
            (a, t[:, 2]),  # tn
        )
        xs, ys = pairs[ci]
        pref = prefp.tile([P, SGB, D], f32, tag="pref")
        nc.vector._custom_dve(SQDIFF_OP, out=pref[:], in0=xs, in1=ys, s0=EPS)
        # prefix at each 512-boundary -> d2p; tail takes adjacent diffs
        nc.scalar.copy(
            out=d2p[:, ci, s, 1 : SGB + 1],
            in_=pref[:, :, D - 1 : D].rearrange("p a b -> p (a b)"),
        )

    # ---- schedule -------------------------------------------------------
    # Sync queue: x-tiles with the six apn pieces interleaved early.
    # GpSimd queue: gathers in 4-block batches, sg1 gated on a mid-kernel
    # label reload so the Q7 descriptor ring drains instead of stalling.
    apn_sched = {0: (0, 0), 1: (0, 1), 2: (0, 2), 4: (1, 0), 6: (1, 1), 8: (1, 2)}
    gather_sched = {
        0: (exa, la_t, range(0, 4), 0),
        1: (exa, la_t, range(4, SGB), 0),
        2: (exn, ln_t, range(0, 4), 0),
        3: (exn, ln_t, range(4, SGB), 0),
        6: (exa, la2_t, range(SGB, 12), SGB),
        7: (exa, la2_t, range(12, NB), SGB),
        8: (exn, ln2_t, range(SGB, 12), SGB),
        9: (exn, ln2_t, range(12, NB), SGB),
    }
    pair_sched = {
        3: [(0, 4)],
        4: [(0, 0)],
        5: [(0, 5), (0, 1)],
        6: [(0, 2)],
        7: [(0, 3), (1, 4)],
        8: [(1, 0), (1, 5)],
        9: [(1, 1), (1, 2)],
        10: [(1, 3)],
    }
    for i in range(NB):
        emit_xtile(i)
        if i in apn_sched:
            emit_apn_load(*apn_sched[i])
        if i == 2:
            emit_exf_stores()
        if i == 5:
            nc.sync.dma_start(out=la2_t[:], in_=la[:, SGB:NB])
            nc.sync.dma_start(out=ln2_t[:], in_=ln_[:, SGB:NB])
        if i in gather_sched:
            emit_gathers(*gather_sched[i])
        for s, ci in pair_sched.get(i, []):
            emit_pair(s, ci)

    # ---- tail ----
    part = sing.tile([P, 4], f32)
    logs = sing.tile([P, NR], f32)
    nc.scalar.activation(out=logs[:], in_=sums[:], func=Act.Ln)
    nc.vector.reduce_sum(out=part[:, 0:1], in_=logs[:], axis=AX.X)
    nc.vector.reduce_sum(out=part[:, 1:2], in_=lblX[:], axis=AX.X)

    # adjacent diffs of the per-supergroup prefixes -> block square-distances
    d2f = sing.tile([P, 6, NSG, SGB], f32)
    nc.vector.tensor_tensor(
        out=d2f[:], in0=d2p[:, :, :, 1 : SGB + 1], in1=d2p[:, :, :, 0:SGB],
        op=Alu.subtract,
    )
    dd = sing.tile([P, 6, NB], f32)
    nc.scalar.activation(
        out=dd[:].rearrange("p c n -> p (c n)"),
        in_=d2f[:].rearrange("p c g b -> p (c g b)"),
        func=Act.Sqrt,
    )

    x1 = sing.tile([P, NB], f32)
    m1 = sing.tile([P, NB], f32)
    c1 = sing.tile([P, NB], f32)
    x2 = sing.tile([P, NB], f32)
    c2 = sing.tile([P, NB], f32)
    x3 = sing.tile([P, NB], f32)
    t3 = sing.tile([P, NB], f32)
    ca = sing.tile([P, 1], f32)
    cb = sing.tile([P, 1], f32)

    # c1 = (dr1 - dn1 > 0) ? (dr1 - dn1 + MARGIN2) : 0
    nc.vector.tensor_tensor(out=x1[:], in0=dd[:, 0, :], in1=dd[:, 1, :], op=Alu.subtract)
    nc.vector.tensor_scalar(
        out=m1[:], in0=x1[:], scalar1=0.0, scalar2=None, op0=Alu.is_gt
    )
    nc.vector.scalar_tensor_tensor(
        out=c1[:], in0=x1[:], scalar=MARGIN2, in1=m1[:],
        op0=Alu.add, op1=Alu.mult, accum_out=ca[:],
    )
    # c2 = relu(dn2 - dr2)
    nc.vector.tensor_tensor(out=x2[:], in0=dd[:, 3, :], in1=dd[:, 2, :], op=Alu.subtract)
    nc.vector.tensor_scalar(
        out=c2[:], in0=x2[:], scalar1=0.0, scalar2=None,
        op0=Alu.max, op1=Alu.add, accum_out=cb[:],
    )
    # t = relu(tp - tn)
    nc.vector.tensor_tensor(out=x3[:], in0=dd[:, 4, :], in1=dd[:, 5, :], op=Alu.subtract)
    nc.vector.tensor_scalar(
        out=t3[:], in0=x3[:], scalar1=0.0, scalar2=None,
        op0=Alu.max, op1=Alu.add, accum_out=part[:, 3:4],
    )
    nc.vector.tensor_tensor(out=part[:, 2:3], in0=ca[:], in1=cb[:], op=Alu.add)
    nc.sync.dma_start(out=pd[:], in_=part[:])


_COMPILED = None


def _build():
    global _COMPILED
    if _COMPILED is not None:
        return _COMPILED
    nc = bacc.Bacc(
        "TRN2",
        target_bir_lowering=False,
        debug=False,
        enable_asserts=False,
        num_devices=NCORES,
    )
    ins = {
        "xout": nc.dram_tensor("xout", [NB, P, 3, C], f32, kind="ExternalInput").ap(),
        "apn": nc.dram_tensor(
            "apn", [NSG, P, 3, SGB, D], f32, kind="ExternalInput"
        ).ap(),
        "exem": nc.dram_tensor("exem", [C, D], f32, kind="ExternalInput").ap(),
        "lab_a": nc.dram_tensor("lab_a", [P, NB], i32, kind="ExternalInput").ap(),
        "lab_n": nc.dram_tensor("lab_n", [P, NB], i32, kind="ExternalInput").ap(),
        "laf": nc.dram_tensor("laf", [P, NB], f32, kind="ExternalInput").ap(),
        "lnf": nc.dram_tensor("lnf", [P, NB], f32, kind="ExternalInput").ap(),
    }
    outs = {
        "partials": nc.dram_tensor("partials", [P, 4], f32, kind="ExternalOutput").ap()
    }
    with tile.TileContext(nc) as tc:
        _emit(tc, outs, ins)
    nc.compile()
    _COMPILED = nc
    return nc


def _in_maps(anchor, positive, negative, outputs, labels_anchor, labels_neg, exemplars):
    anchor = np.asarray(anchor, np.float32)
    positive = np.asarray(positive, np.float32)
    negative = np.asarray(negative, np.float32)
    outputs = np.asarray(outputs, np.float32)
    exemplars = np.ascontiguousarray(np.asarray(exemplars, np.float32))
    la_all = np.asarray(labels_anchor).astype(np.int64)
    ln_all = np.asarray(labels_neg).astype(np.int64)

    maps = []
    for k in range(NCORES):
        sl_ = slice(k * BS, (k + 1) * BS)
        la, ln = la_all[sl_], ln_all[sl_]
        # [3, BS, C] -> [NB, P, 3, C]
        xo3 = np.stack(
            [
                outputs[k * BS : (k + 1) * BS],
                outputs[B + k * BS : B + (k + 1) * BS],
                outputs[2 * B + k * BS : 2 * B + (k + 1) * BS],
            ],
            axis=0,
        )
        xo = np.ascontiguousarray(xo3.reshape(3, NB, P, C).transpose(1, 2, 0, 3))
        # a/p/n [BS, D] -> [NSG, P, 3, SGB, D]
        apn3 = np.stack([anchor[sl_], positive[sl_], negative[sl_]], axis=0)
        apn = np.ascontiguousarray(
            apn3.reshape(3, NSG, SGB, P, D).transpose(1, 3, 0, 2, 4)
        )
        maps.append(
            {
                "xout": xo,
                "apn": apn,
                "exem": exemplars,
                "lab_a": np.ascontiguousarray(la.reshape(NB, P).T.astype(np.int32)),
                "lab_n": np.ascontiguousarray(ln.reshape(NB, P).T.astype(np.int32)),
                "laf": np.ascontiguousarray(la.reshape(NB, P).T.astype(np.float32)),
                "lnf": np.ascontiguousarray(
                    (ln + 2000).reshape(NB, P).T.astype(np.float32)
                ),
            }
        )
    return maps


def _combine(results):
    S = np.zeros(4, dtype=np.float64)
    for r in results:
        S += r["partials"].astype(np.float64).sum(axis=0)
    loss_softmax = (S[0] - S[1]) / (3 * B)
    loss_center = S[2]
    loss_triplet = S[3]
    loss_total = loss_softmax + 0.01 * loss_center + LAMBDA * loss_triplet
    return (
        np.float32(loss_total),
        np.float32(loss_triplet),
        np.float32(loss_softmax),
        np.float32(loss_center),
    )


def kernel(anchor, positive, negative, outputs, labels_anchor, labels_neg, exemplars):
    global LAST_RESULTS
    nc = _build()
    maps = _in_maps(
        anchor, positive, negative, outputs, labels_anchor, labels_neg, exemplars
    )
    res = run_bass_kernel_spmd(nc, maps, core_ids=list(range(NCORES)))
    LAST_RESULTS = res
    return _combine(res.results)
